# revision 21
# baseline (speedup 1.0000x reference)
"""Trainium2 Bass kernel for ConditionalFeedForward (MoE routed FFN).

Problem: M=2048 tokens, D=1024, I=2048, E=8 experts, TOPK=2.
out[t, s] = FFN_{e}(x[t]) with e = expert_indices[t, s], where
FFN_e(x) = (silu(x @ w1_e.T) * (x @ w3_e.T)) @ w2_e.T  (w13 = [w1; w3]).

Strategy (expert parallelism, 8 experts -> 8 cores):
 - Host routes (token, slot) pairs to the core owning the selected expert,
   pads each core's token batch to a common capacity C, and transposes
   activations so features live on SBUF partitions and tokens on the free
   dim.  No device collectives needed: the "all-to-all" is a host gather
   and scatter around one SPMD kernel launch.
 - All matmul operands are cast to bf16 on the host: halves the weight
   DMA stream (12.6 MB/core instead of 25.2) and shrinks LDWEIGHTS to
   1 cycle/row so it pipelines fully behind each 504-row MULT.
 - Per core: hT = w13_e @ x_eT on PE, g = silu(h1) * h3 on ACT+DVE (g in
   bf16), outT = w2_e.T-contract on PE, fp32 out DMA.
"""

import os

import numpy as np
from ml_dtypes import bfloat16

import concourse.bass as bass
import concourse.tile as tile
from concourse import bacc, mybir
from concourse.bass_utils import run_bass_kernel_spmd

M, D, I, E, TOPK = 2048, 1024, 2048, 8, 2
P = 128
KD = D // P            # 8   k-tiles over D (mm1 contraction)
NI2 = (2 * I) // P     # 32  n-tiles over 2I (mm1 output rows)
NPAIR = NI2 // 2       # 16  (x1, x3) pairs
KI = I // P            # 16  k-tiles over I (mm2 contraction)
ND = D // P            # 8   d-tiles over D (mm2 output rows)

F32 = mybir.dt.float32
BF16 = mybir.dt.bfloat16

# exec time of the most recent launch (ns), populated when BASS_TRACE=1
LAST_EXEC_TIME_NS = None

_program_cache = {}


def _chunks_for(C):
    """Split C token-columns into matmul moving-dim chunks (<=512 each)."""
    n_ch = -(-C // 512)
    base = -(-C // (n_ch * 32)) * 32
    chunks = []
    off = 0
    while off < C:
        cn = min(base, C - off)
        chunks.append((off, cn))
        off += cn
    return tuple(chunks)


def _build_program(C):
    chunks = _chunks_for(C)
    nc = bacc.Bacc(
        "TRN2",
        target_bir_lowering=False,
        debug=False,
        enable_asserts=False,
        num_devices=E,
    )

    # x: partition-major fused layout [P, KD, C] (4KB/partition runs)
    # w13: (x1, x3) row-block PAIRS fused per partition row -> 4KB runs
    # w2: two d-blocks fused per row -> 8KB runs
    xT_d = nc.dram_tensor("xT", (P, KD, C), BF16, kind="ExternalInput").ap()
    w13_d = nc.dram_tensor(
        "w13t", (NPAIR, P, 2 * KD * P), BF16, kind="ExternalInput"
    ).ap()
    w2_d = nc.dram_tensor(
        "w2t", (ND // 2, P, 2 * KI * P), BF16, kind="ExternalInput"
    ).ap()
    out_d = nc.dram_tensor("outT", (ND, P, C), F32, kind="ExternalOutput").ap()

    with tile.TileContext(nc) as tc:
        with (
            tc.tile_pool(name="xg", bufs=1) as xg_pool,
            tc.tile_pool(name="wt", bufs=3) as w_pool,
            tc.tile_pool(name="tmp", bufs=4) as tmp_pool,
            tc.tile_pool(name="ps", bufs=8, space="PSUM") as ps_pool,
        ):
            W13_BUFS = 8
            W13_AHEAD = W13_BUFS // 2 - 1
            w13_buf = {}

            def issue_w13(pr):
                wA = w_pool.tile(
                    [P, KD * P], BF16, tag="w13", name="w13", bufs=W13_BUFS
                )
                nc.sync.dma_start(wA[:], w13_d[pr][:, : KD * P])
                wB = w_pool.tile(
                    [P, KD * P], BF16, tag="w13", name="w13", bufs=W13_BUFS
                )
                nc.sync.dma_start(wB[:], w13_d[pr][:, KD * P :])
                w13_buf[pr] = (wA, wB)

            W2F_BUFS = 3
            w2f_buf = {}

            def issue_w2f(j):
                wDD = w_pool.tile(
                    [P, 2 * KI * P], BF16, tag="w2f", name="w2f", bufs=W2F_BUFS
                )
                nc.sync.dma_start(wDD[:], w2_d[j])
                w2f_buf[j] = wDD

            # ---- PE p-state management.  TRN2's PE clock ramps 0.65 -> 1.2
            # -> 2.4 GHz over ~3us of CONTINUOUS activity and resets to a
            # low p-state after even sub-microsecond idle gaps (measured:
            # a 1.5us gap put the next several us of matmuls at 1.2 GHz).
            # So junk matmuls (a) bridge from program start until the first
            # real operands land and (b) fill the data-starved holes while
            # the x stream trickles in, keeping the PE nominally busy and
            # the clock at 2.4 GHz for every real matmul.
            warm = tmp_pool.tile([P, 2 * P], BF16, tag="warm", name="warm")
            nc.gpsimd.memset(warm[:], 0.0)
            psW = ps_pool.tile([P, 512], F32, tag="ps", name="ps")

            def fillers(n):
                for _ in range(n):
                    nc.tensor.matmul(
                        psW[:, :P], warm[:, :P], warm[:, P:],
                        start=True, stop=True,
                    )

            fillers(38)

            # startup order: the first matmul needs only x[k=0] and pair-0
            # wA.  x streams per-k (region-level semaphores -> no wait on
            # the full tensor), in PE consumption order, all down the sync
            # queue which demonstrably sustains the full 360 GB/s.  Pair 0
            # consumes its A and B halves interleaved per k (see below), so
            # x arrival (0.36us/tile) stays ahead of PE consumption
            # (0.42us/tile) and the startup has no exposed stalls.
            xall = xg_pool.tile([P, KD * C], BF16, tag="x", name="x")
            x_tiles = [xall[:, k * C : (k + 1) * C] for k in range(KD)]
            wA0 = w_pool.tile([P, KD * P], BF16, tag="w130", name="w130", bufs=1)
            wB0 = w_pool.tile([P, KD * P], BF16, tag="w131", name="w131", bufs=1)
            nc.sync.dma_start(
                xall[:, : 2 * C].rearrange("p (k c) -> p k c", c=C),
                xT_d[:, 0:2, :],
            )
            nc.sync.dma_start(wA0[:], w13_d[0][:, : KD * P])
            nc.sync.dma_start(wB0[:], w13_d[0][:, KD * P :])
            for k in range(2, KD, 2):
                nc.sync.dma_start(
                    xall[:, k * C : (k + 2) * C].rearrange(
                        "p (k c) -> p k c", c=C
                    ),
                    xT_d[:, k : k + 2, :],
                )
            w13_buf[0] = (wA0, wB0)
            for pr in range(1, 1 + W13_AHEAD):
                issue_w13(pr)
            # fillers inside pair 0, sized to the DMA-arrival gaps measured
            # in the trace (early DMA engines ramp up slowly, so the stream
            # runs ~30% behind the steady-state bandwidth model)
            PAIR0_FILL = {(0, "B"): 9, (2, "A"): 3, (4, "A"): 3, (6, "A"): 3}
            PAIR1_FILL = 8

            g_tiles = [
                xg_pool.tile([P, C], BF16, tag=f"g{ki}", name=f"g{ki}")
                for ki in range(KI)
            ]

            # ---- mm1 + silu*gate: process (x1, x3) row-block pairs ----
            for pr in range(NPAIR):
                nxt = pr + 1 + W13_AHEAD
                if nxt < NPAIR:
                    issue_w13(nxt)
                elif nxt - NPAIR < min(W2F_BUFS, ND // 2):
                    issue_w2f(nxt - NPAIR)
                wA, wB = w13_buf.pop(pr)
                for c0, cn in chunks:
                    psA = ps_pool.tile([P, 512], F32, tag="ps", name="ps")[:, :cn]
                    psB = ps_pool.tile([P, 512], F32, tag="ps", name="ps")[:, :cn]
                    if pr == 0:
                        # interleave A/B per k so each freshly-landed x tile
                        # feeds 2 matmuls (~0.42us) vs its ~0.36us DMA time
                        for k in range(KD):
                            if (k, "A") in PAIR0_FILL:
                                fillers(PAIR0_FILL[(k, "A")])
                            nc.tensor.matmul(
                                psA, wA[:, k * P : (k + 1) * P],
                                x_tiles[k][:, c0 : c0 + cn],
                                start=(k == 0), stop=(k == KD - 1),
                            )
                            if (k, "B") in PAIR0_FILL:
                                fillers(PAIR0_FILL[(k, "B")])
                            nc.tensor.matmul(
                                psB, wB[:, k * P : (k + 1) * P],
                                x_tiles[k][:, c0 : c0 + cn],
                                start=(k == 0), stop=(k == KD - 1),
                            )
                    else:
                        if pr == 1:
                            fillers(PAIR1_FILL)
                        for k in range(KD):
                            nc.tensor.matmul(
                                psA, wA[:, k * P : (k + 1) * P],
                                x_tiles[k][:, c0 : c0 + cn],
                                start=(k == 0), stop=(k == KD - 1),
                            )
                        for k in range(KD):
                            nc.tensor.matmul(
                                psB, wB[:, k * P : (k + 1) * P],
                                x_tiles[k][:, c0 : c0 + cn],
                                start=(k == 0), stop=(k == KD - 1),
                            )
                    s = tmp_pool.tile([P, 512], F32, tag="s", name="s")[:, :cn]
                    nc.scalar.activation(s, psA, mybir.ActivationFunctionType.Silu)
                    nc.vector.tensor_mul(
                        out=g_tiles[pr][:, c0 : c0 + cn],
                        in0=s,
                        in1=psB,
                    )

            # ---- mm2: outT[d-block] = sum_ki w2T-tile @ g ----
            for d in range(ND):
                if d % 2 == 0:
                    j = d // 2
                    if j + W2F_BUFS < ND // 2:
                        issue_w2f(j + W2F_BUFS)
                wDD = w2f_buf[d // 2]
                wD = wDD[:, (d % 2) * KI * P : (d % 2 + 1) * KI * P]
                if d % 2 == 1:
                    w2f_buf.pop(d // 2)
                # last d-block: taper the column chunks (1/2, 1/4, 1/8, 1/8)
                # so the copy+DMA of early pieces overlaps the remaining
                # matmuls and the final exposed copy+config+DMA+sem chain
                # covers only ~62 columns.
                if d == ND - 1:
                    dchunks = []
                    for c0, cn in chunks:
                        rem = cn
                        off = c0
                        for frac in (2, 4):
                            piece = cn // frac - (cn // frac) % 2
                            dchunks.append((off, piece))
                            off += piece
                            rem -= piece
                        dchunks.append((off, rem))
                else:
                    dchunks = chunks
                for c0, cn in dchunks:
                    psO = ps_pool.tile([P, 512], F32, tag="ps", name="ps")[:, :cn]
                    for ki in range(KI):
                        nc.tensor.matmul(
                            psO,
                            wD[:, ki * P : (ki + 1) * P],
                            g_tiles[ki][:, c0 : c0 + cn],
                            start=(ki == 0),
                            stop=(ki == KI - 1),
                        )
                    ot = tmp_pool.tile([P, 512], F32, tag="o", name="o", bufs=8)[
                        :, :cn
                    ]
                    nc.vector.tensor_copy(ot, psO)
                    nc.sync.dma_start(out_d[d][:, c0 : c0 + cn], ot)

    nc.compile()
    return nc


def _get_program(C):
    if C not in _program_cache:
        _program_cache[C] = _build_program(C)
    return _program_cache[C]


def _ensure_ntff_hook():
    """Provide antenv.axon_hooks if the image lacks it, so trace=True works."""
    import sys
    import types

    try:
        import antenv.axon_hooks  # noqa: F401

        return
    except ImportError:
        pass
    try:
        import antenv
        from trn_agent_boot.trn_boot import _ntff_profile_via_ctypes

        mod = types.ModuleType("antenv.axon_hooks")
        state = {"hook": None}
        mod.set_axon_ntff_profile_hook = lambda h: state.__setitem__("hook", h)
        mod.get_axon_ntff_profile_hook = lambda: state["hook"]
        sys.modules["antenv.axon_hooks"] = mod
        antenv.axon_hooks = mod
        mod.set_axon_ntff_profile_hook(
            _ntff_profile_via_ctypes("/opt/axon/libaxon_pjrt.so")
        )
    except Exception:
        pass


def kernel(x, w13, w2, expert_indices):
    global LAST_EXEC_TIME_NS
    x = np.asarray(x, dtype=np.float32)
    w13 = np.asarray(w13, dtype=np.float32)
    w2 = np.asarray(w2, dtype=np.float32)
    idx = np.asarray(expert_indices)
    idx32 = idx.astype(np.int64)

    m, d_model = x.shape
    e, two_i, _ = w13.shape
    inter = w2.shape[2]
    topk = idx.shape[1]
    assert (m, d_model, e, two_i, inter, topk) == (M, D, E, 2 * I, I, TOPK)

    # ---- host routing: unique (token, expert) work items per expert ----
    # A token picking the same expert in both slots computes the FFN once;
    # the result is scattered to every matching slot.
    tok_unique = [
        np.unique(np.concatenate([np.nonzero(idx32[:, s] == ei)[0] for s in range(topk)]))
        for ei in range(E)
    ]
    max_cnt = max(len(u) for u in tok_unique)
    C = max(256, int(-(-max_cnt // 2) * 2))

    nc = _get_program(C)

    in_maps = []
    for ei in range(E):
        tok_ids = tok_unique[ei]
        cnt = len(tok_ids)

        xg = np.zeros((C, D), dtype=np.float32)
        xg[:cnt] = x[tok_ids]
        xT = np.ascontiguousarray(
            xg.T.reshape(KD, P, C).transpose(1, 0, 2)
        ).astype(bfloat16)                           # [p, k, c]

        A4 = w13[ei].reshape(NI2, P, KD, P)          # [n, c, k, p]
        w13t = A4.transpose(0, 3, 2, 1).reshape(NI2, P, KD * P)
        w13p = np.ascontiguousarray(
            np.concatenate([w13t[:NPAIR], w13t[NPAIR:]], axis=2)
        ).astype(bfloat16)                           # [pair, p, 2*KD*P]
        B4 = w2[ei].reshape(ND, P, KI, P)            # [d, c, ki, p]
        w2t = B4.transpose(0, 3, 2, 1).reshape(ND, P, KI * P)
        w2p = np.ascontiguousarray(
            w2t.reshape(ND // 2, 2, P, KI * P).transpose(0, 2, 1, 3).reshape(
                ND // 2, P, 2 * KI * P
            )
        ).astype(bfloat16)                           # [dpair, p, 2*KI*P]

        in_maps.append({"xT": xT, "w13t": w13p, "w2t": w2p})

    trace = bool(os.environ.get("BASS_TRACE"))
    if trace:
        _ensure_ntff_hook()
    res = run_bass_kernel_spmd(nc, in_maps, core_ids=list(range(E)), trace=trace)
    LAST_EXEC_TIME_NS = res.exec_time_ns

    # ---- host scatter: copy each expert's outputs to all matching slots ----
    out = np.empty((M, topk, D), dtype=np.float32)
    for ei in range(E):
        outT = res.results[ei]["outT"].reshape(D, C)
        oe = outT[:, : len(tok_unique[ei])].T        # [cnt, D]
        for s in range(topk):
            sel = np.nonzero(idx32[:, s] == ei)[0]
            out[sel, s] = oe[np.searchsorted(tok_unique[ei], sel)]

    return out


# revision 22
# speedup vs baseline: 1.1730x; 1.1730x over previous
"""Trainium2 Bass kernel for ConditionalFeedForward (MoE routed FFN).

Problem: M=2048 tokens, D=1024, I=2048, E=8 experts, TOPK=2.
out[t, s] = FFN_{e}(x[t]) with e = expert_indices[t, s], where
FFN_e(x) = (silu(x @ w1_e.T) * (x @ w3_e.T)) @ w2_e.T  (w13 = [w1; w3]).

Strategy (expert parallelism, 8 experts -> 8 cores):
 - Host routes (token, slot) pairs to the core owning the selected expert,
   pads each core's token batch to a common capacity C, and transposes
   activations so features live on SBUF partitions and tokens on the free
   dim.  No device collectives needed: the "all-to-all" is a host gather
   and scatter around one SPMD kernel launch.
 - All matmul operands are cast to bf16 on the host: halves the weight
   DMA stream (12.6 MB/core instead of 25.2) and shrinks LDWEIGHTS to
   1 cycle/row so it pipelines fully behind each 504-row MULT.
 - Per core: hT = w13_e @ x_eT on PE, g = silu(h1) * h3 on ACT+DVE (g in
   bf16), outT = w2_e.T-contract on PE, fp32 out DMA.
"""

import os

import numpy as np
from ml_dtypes import bfloat16

import concourse.bass as bass
import concourse.tile as tile
from concourse import bacc, mybir
from concourse.bass_utils import run_bass_kernel_spmd

M, D, I, E, TOPK = 2048, 1024, 2048, 8, 2
P = 128
KD = D // P            # 8   k-tiles over D (mm1 contraction)
NI2 = (2 * I) // P     # 32  n-tiles over 2I (mm1 output rows)
NPAIR = NI2 // 2       # 16  (x1, x3) pairs
KI = I // P            # 16  k-tiles over I (mm2 contraction)
ND = D // P            # 8   d-tiles over D (mm2 output rows)

F32 = mybir.dt.float32
BF16 = mybir.dt.bfloat16

# exec time of the most recent launch (ns), populated when BASS_TRACE=1
LAST_EXEC_TIME_NS = None

_program_cache = {}


def _chunks_for(C):
    """Split C token-columns into matmul moving-dim chunks (<=512 each)."""
    n_ch = -(-C // 512)
    base = -(-C // (n_ch * 32)) * 32
    chunks = []
    off = 0
    while off < C:
        cn = min(base, C - off)
        chunks.append((off, cn))
        off += cn
    return tuple(chunks)


def _build_program(C):
    chunks = _chunks_for(C)
    nc = bacc.Bacc(
        "TRN2",
        target_bir_lowering=False,
        debug=False,
        enable_asserts=False,
        num_devices=E,
    )

    # x: partition-major fused layout [P, KD, C] (4KB/partition runs)
    # w13: (x1, x3) row-block PAIRS fused per partition row -> 4KB runs
    # w2: two d-blocks fused per row -> 8KB runs
    xT_d = nc.dram_tensor("xT", (P, KD, C), BF16, kind="ExternalInput").ap()
    w13_d = nc.dram_tensor(
        "w13t", (NPAIR, P, 2 * KD * P), BF16, kind="ExternalInput"
    ).ap()
    w2_d = nc.dram_tensor(
        "w2t", (ND // 2, P, 2 * KI * P), BF16, kind="ExternalInput"
    ).ap()
    out_d = nc.dram_tensor("outT", (ND, P, C), F32, kind="ExternalOutput").ap()

    with tile.TileContext(nc) as tc:
        with (
            tc.tile_pool(name="xg", bufs=1) as xg_pool,
            tc.tile_pool(name="wt", bufs=3) as w_pool,
            tc.tile_pool(name="tmp", bufs=4) as tmp_pool,
            tc.tile_pool(name="ps", bufs=8, space="PSUM") as ps_pool,
        ):
            W13_BUFS = 8
            W13_AHEAD = W13_BUFS // 2 - 1
            w13_buf = {}

            def issue_w13(pr):
                wA = w_pool.tile(
                    [P, KD * P], BF16, tag="w13", name="w13", bufs=W13_BUFS
                )
                nc.sync.dma_start(wA[:], w13_d[pr][:, : KD * P])
                wB = w_pool.tile(
                    [P, KD * P], BF16, tag="w13", name="w13", bufs=W13_BUFS
                )
                nc.sync.dma_start(wB[:], w13_d[pr][:, KD * P :])
                w13_buf[pr] = (wA, wB)

            W2F_BUFS = 3
            w2f_buf = {}

            def issue_w2f(j):
                wDD = w_pool.tile(
                    [P, 2 * KI * P], BF16, tag="w2f", name="w2f", bufs=W2F_BUFS
                )
                nc.sync.dma_start(wDD[:], w2_d[j])
                w2f_buf[j] = wDD

            # ---- PE p-state management.  TRN2's PE clock ramps 0.65 -> 1.2
            # -> 2.4 GHz over ~3us of CONTINUOUS activity and resets to a
            # low p-state after even sub-microsecond idle gaps (measured:
            # a 1.5us gap put the next several us of matmuls at 1.2 GHz).
            # So junk matmuls (a) bridge from program start until the first
            # real operands land and (b) fill the data-starved holes while
            # the x stream trickles in, keeping the PE nominally busy and
            # the clock at 2.4 GHz for every real matmul.
            warm = tmp_pool.tile([P, 2 * P], BF16, tag="warm", name="warm")
            nc.gpsimd.memset(warm[:], 0.0)
            psW = ps_pool.tile([P, 512], F32, tag="ps", name="ps")

            def fillers(n):
                for _ in range(n):
                    nc.tensor.matmul(
                        psW[:, :P], warm[:, :P], warm[:, P:],
                        start=True, stop=True,
                    )

            fillers(38)

            # startup order: the first matmul needs only x[k=0] and pair-0
            # wA.  x streams per-k (region-level semaphores -> no wait on
            # the full tensor), in PE consumption order, all down the sync
            # queue which demonstrably sustains the full 360 GB/s.  Pair 0
            # consumes its A and B halves interleaved per k (see below), so
            # x arrival (0.36us/tile) stays ahead of PE consumption
            # (0.42us/tile) and the startup has no exposed stalls.
            xall = xg_pool.tile([P, KD * C], BF16, tag="x", name="x")
            x_tiles = [xall[:, k * C : (k + 1) * C] for k in range(KD)]
            wA0 = w_pool.tile([P, KD * P], BF16, tag="w130", name="w130", bufs=1)
            wB0 = w_pool.tile([P, KD * P], BF16, tag="w131", name="w131", bufs=1)
            nc.sync.dma_start(xall[:, :C], xT_d[:, 0, :])
            nc.sync.dma_start(wA0[:], w13_d[0][:, : KD * P])
            nc.sync.dma_start(wB0[:], w13_d[0][:, KD * P :])
            for k in range(1, KD):
                nc.sync.dma_start(
                    xall[:, k * C : (k + 1) * C], xT_d[:, k, :]
                )
            w13_buf[0] = (wA0, wB0)
            for pr in range(1, 1 + W13_AHEAD):
                issue_w13(pr)
            # fillers inside pair 0, sized to the DMA-arrival gaps measured
            # in the trace (early DMA engines ramp up slowly, so the stream
            # runs ~30% behind the steady-state bandwidth model)
            PAIR0_FILL = {(0, "B"): 8, (1, "A"): 5, (2, "A"): 3, (3, "A"): 2}
            PAIR1_FILL = 8

            g_tiles = [
                xg_pool.tile([P, C], BF16, tag=f"g{ki}", name=f"g{ki}")
                for ki in range(KI)
            ]

            # ---- mm1 + silu*gate: process (x1, x3) row-block pairs ----
            for pr in range(NPAIR):
                nxt = pr + 1 + W13_AHEAD
                if nxt < NPAIR:
                    issue_w13(nxt)
                elif nxt - NPAIR < min(W2F_BUFS, ND // 2):
                    issue_w2f(nxt - NPAIR)
                wA, wB = w13_buf.pop(pr)
                for c0, cn in chunks:
                    psA = ps_pool.tile([P, 512], F32, tag="ps", name="ps")[:, :cn]
                    psB = ps_pool.tile([P, 512], F32, tag="ps", name="ps")[:, :cn]
                    if pr == 0:
                        # interleave A/B per k so each freshly-landed x tile
                        # feeds 2 matmuls (~0.42us) vs its ~0.36us DMA time
                        for k in range(KD):
                            if (k, "A") in PAIR0_FILL:
                                fillers(PAIR0_FILL[(k, "A")])
                            nc.tensor.matmul(
                                psA, wA[:, k * P : (k + 1) * P],
                                x_tiles[k][:, c0 : c0 + cn],
                                start=(k == 0), stop=(k == KD - 1),
                            )
                            if (k, "B") in PAIR0_FILL:
                                fillers(PAIR0_FILL[(k, "B")])
                            nc.tensor.matmul(
                                psB, wB[:, k * P : (k + 1) * P],
                                x_tiles[k][:, c0 : c0 + cn],
                                start=(k == 0), stop=(k == KD - 1),
                            )
                    else:
                        if pr == 1:
                            fillers(PAIR1_FILL)
                        for k in range(KD):
                            nc.tensor.matmul(
                                psA, wA[:, k * P : (k + 1) * P],
                                x_tiles[k][:, c0 : c0 + cn],
                                start=(k == 0), stop=(k == KD - 1),
                            )
                        for k in range(KD):
                            nc.tensor.matmul(
                                psB, wB[:, k * P : (k + 1) * P],
                                x_tiles[k][:, c0 : c0 + cn],
                                start=(k == 0), stop=(k == KD - 1),
                            )
                    s = tmp_pool.tile([P, 512], F32, tag="s", name="s")[:, :cn]
                    nc.scalar.activation(s, psA, mybir.ActivationFunctionType.Silu)
                    nc.vector.tensor_mul(
                        out=g_tiles[pr][:, c0 : c0 + cn],
                        in0=s,
                        in1=psB,
                    )

            # ---- mm2: outT[d-block] = sum_ki w2T-tile @ g ----
            for d in range(ND):
                if d % 2 == 0:
                    j = d // 2
                    if j + W2F_BUFS < ND // 2:
                        issue_w2f(j + W2F_BUFS)
                wDD = w2f_buf[d // 2]
                wD = wDD[:, (d % 2) * KI * P : (d % 2 + 1) * KI * P]
                if d % 2 == 1:
                    w2f_buf.pop(d // 2)
                # last d-block: taper the column chunks (1/2, 1/4, 1/8, 1/8)
                # so the copy+DMA of early pieces overlaps the remaining
                # matmuls and the final exposed copy+config+DMA+sem chain
                # covers only ~62 columns.
                if d == ND - 1:
                    dchunks = []
                    for c0, cn in chunks:
                        rem = cn
                        off = c0
                        for frac in (2, 4):
                            piece = cn // frac - (cn // frac) % 2
                            dchunks.append((off, piece))
                            off += piece
                            rem -= piece
                        dchunks.append((off, rem))
                else:
                    dchunks = chunks
                for c0, cn in dchunks:
                    psO = ps_pool.tile([P, 512], F32, tag="ps", name="ps")[:, :cn]
                    for ki in range(KI):
                        nc.tensor.matmul(
                            psO,
                            wD[:, ki * P : (ki + 1) * P],
                            g_tiles[ki][:, c0 : c0 + cn],
                            start=(ki == 0),
                            stop=(ki == KI - 1),
                        )
                    ot = tmp_pool.tile([P, 512], F32, tag="o", name="o", bufs=8)[
                        :, :cn
                    ]
                    nc.vector.tensor_copy(ot, psO)
                    nc.sync.dma_start(out_d[d][:, c0 : c0 + cn], ot)

    nc.compile()
    return nc


def _get_program(C):
    if C not in _program_cache:
        _program_cache[C] = _build_program(C)
    return _program_cache[C]


def _ensure_ntff_hook():
    """Provide antenv.axon_hooks if the image lacks it, so trace=True works."""
    import sys
    import types

    try:
        import antenv.axon_hooks  # noqa: F401

        return
    except ImportError:
        pass
    try:
        import antenv
        from trn_agent_boot.trn_boot import _ntff_profile_via_ctypes

        mod = types.ModuleType("antenv.axon_hooks")
        state = {"hook": None}
        mod.set_axon_ntff_profile_hook = lambda h: state.__setitem__("hook", h)
        mod.get_axon_ntff_profile_hook = lambda: state["hook"]
        sys.modules["antenv.axon_hooks"] = mod
        antenv.axon_hooks = mod
        mod.set_axon_ntff_profile_hook(
            _ntff_profile_via_ctypes("/opt/axon/libaxon_pjrt.so")
        )
    except Exception:
        pass


def kernel(x, w13, w2, expert_indices):
    global LAST_EXEC_TIME_NS
    x = np.asarray(x, dtype=np.float32)
    w13 = np.asarray(w13, dtype=np.float32)
    w2 = np.asarray(w2, dtype=np.float32)
    idx = np.asarray(expert_indices)
    idx32 = idx.astype(np.int64)

    m, d_model = x.shape
    e, two_i, _ = w13.shape
    inter = w2.shape[2]
    topk = idx.shape[1]
    assert (m, d_model, e, two_i, inter, topk) == (M, D, E, 2 * I, I, TOPK)

    # ---- host routing: unique (token, expert) work items per expert ----
    # A token picking the same expert in both slots computes the FFN once;
    # the result is scattered to every matching slot.
    tok_unique = [
        np.unique(np.concatenate([np.nonzero(idx32[:, s] == ei)[0] for s in range(topk)]))
        for ei in range(E)
    ]
    max_cnt = max(len(u) for u in tok_unique)
    C = max(256, int(-(-max_cnt // 2) * 2))

    nc = _get_program(C)

    in_maps = []
    for ei in range(E):
        tok_ids = tok_unique[ei]
        cnt = len(tok_ids)

        xg = np.zeros((C, D), dtype=np.float32)
        xg[:cnt] = x[tok_ids]
        xT = np.ascontiguousarray(
            xg.T.reshape(KD, P, C).transpose(1, 0, 2)
        ).astype(bfloat16)                           # [p, k, c]

        A4 = w13[ei].reshape(NI2, P, KD, P)          # [n, c, k, p]
        w13t = A4.transpose(0, 3, 2, 1).reshape(NI2, P, KD * P)
        w13p = np.ascontiguousarray(
            np.concatenate([w13t[:NPAIR], w13t[NPAIR:]], axis=2)
        ).astype(bfloat16)                           # [pair, p, 2*KD*P]
        B4 = w2[ei].reshape(ND, P, KI, P)            # [d, c, ki, p]
        w2t = B4.transpose(0, 3, 2, 1).reshape(ND, P, KI * P)
        w2p = np.ascontiguousarray(
            w2t.reshape(ND // 2, 2, P, KI * P).transpose(0, 2, 1, 3).reshape(
                ND // 2, P, 2 * KI * P
            )
        ).astype(bfloat16)                           # [dpair, p, 2*KI*P]

        in_maps.append({"xT": xT, "w13t": w13p, "w2t": w2p})

    trace = bool(os.environ.get("BASS_TRACE"))
    if trace:
        _ensure_ntff_hook()
    res = run_bass_kernel_spmd(nc, in_maps, core_ids=list(range(E)), trace=trace)
    LAST_EXEC_TIME_NS = res.exec_time_ns

    # ---- host scatter: copy each expert's outputs to all matching slots ----
    out = np.empty((M, topk, D), dtype=np.float32)
    for ei in range(E):
        outT = res.results[ei]["outT"].reshape(D, C)
        oe = outT[:, : len(tok_unique[ei])].T        # [cnt, D]
        for s in range(topk):
            sel = np.nonzero(idx32[:, s] == ei)[0]
            out[sel, s] = oe[np.searchsorted(tok_unique[ei], sel)]

    return out
